# revision 20
# baseline (speedup 1.0000x reference)
"""Bass/Trainium2 kernel for LinearRowShared4Bit.

y[b,s,o] = sum_i x[b,s,i] * W[o,i] + bias[o]
W[o,i]   = (2*q[o,i]/15 - 1) * norm[o//32, i//32]   (q = 4-bit nibbles)

Sharding: out_features (11008) split 1376-per-core across 8 cores; x replicated.

Host does layout only (transpose / nibble->byte unpack / norm regrouping);
all dequant arithmetic ((q-7.5)*(2*norm/15)) and the matmul run on device.

Device per core (v3 schedule):
  - the per-(k-block, o-group) scales have only 43 distinct columns per core
    (scales are shared by 32 consecutive out features), so the one-hot
    expansion matmul that routes scale rows onto the 128 partitions only
    produces a compact [128, 43] tile per k-tile (~150ns each); the DVE
    dequant reads it through a stride-0 broadcast access pattern,
  - W is dequantized into a resident SBUF tile [128, 32, 1376] fp16
    (11.3 MB) k-tile by k-tile, with per-tile DMAs ordered by criticality,
  - dequant overlaps main compute: the first two m-tiles' PSUM accumulation
    groups are held open across the dequant loop and consume each W k-tile
    as soon as it is ready,
  - the remaining 128-token tiles run the usual 3 o-chunk x 32 k-tile
    accumulation, bias-add on DVE, fp32 rows DMA'd out.
"""

import numpy as np

IN_F = 4096
OUT_F = 11008
N_CORES = 8
O_SH = OUT_F // N_CORES  # 1376
KT = IN_F // 128         # 32 k-tiles
MS = 256                 # tokens per x-slab DMA
WMT = 2                  # m-tiles accumulated during the dequant window

_PROG = {}


def _build(M, O, kt):
    import concourse.mybir as mybir
    import concourse.tile as tile
    from concourse import bacc

    f16, f32, u8 = mybir.dt.float16, mybir.dt.float32, mybir.dt.uint8
    nc = bacc.Bacc("TRN2", target_bir_lowering=False, debug=False,
                   num_devices=N_CORES)
    K = kt * 128
    OG = O // 32  # distinct scale columns (43)
    xT = nc.dram_tensor("xT", (M // 128, 128, K), f16, kind="ExternalInput")
    wq = nc.dram_tensor("wq", (K, O), u8, kind="ExternalInput")
    s2c = nc.dram_tensor("s2c", (kt * 4, OG), f16, kind="ExternalInput")
    ee = nc.dram_tensor("ee", (128, kt * 128), f16, kind="ExternalInput")
    bb = nc.dram_tensor("bb", (128, O), f32, kind="ExternalInput")
    y = nc.dram_tensor("y", (M, O), f32, kind="ExternalOutput")

    chunks = [(o0, min(512, O - o0)) for o0 in range(0, O, 512)]
    ms_cnt = M // MS
    sub, mult = mybir.AluOpType.subtract, mybir.AluOpType.mult

    with tile.TileContext(nc) as tc:
        with (
            tc.tile_pool(name="wres", bufs=32) as wres,
            tc.tile_pool(name="consts", bufs=1) as consts,
            tc.tile_pool(name="qp", bufs=16) as qp,
            tc.tile_pool(name="xp", bufs=4) as xp,
            tc.tile_pool(name="op", bufs=3) as op,
            tc.tile_pool(name="pp", bufs=6, space="PSUM") as pp,
            tc.tile_pool(name="spp", bufs=2, space="PSUM") as spp,
        ):
            # one W tile per k-tile: exact per-tile dependencies, so a
            # consume matmul only waits on its own k-tile's dequant
            w_tiles = [wres.tile([128, O], f16, name=f"w{t}", tag="w")
                       for t in range(kt)]
            scomp = consts.tile([128, kt, OG], f16)

            # PE warm-up: the tensor engine's clock gate (HAM) keeps the PE
            # at half clock until it has seen ~3.4us of *sustained* activity,
            # and re-throttles after ~3.4us of idle. The startup DMAs take
            # ~15us, so burn a solid burst of back-to-back dummy matmuls
            # first; by the time the real matmuls start the PE is at full
            # clock and the idle gap to them is under the re-throttle window.
            # Reads scratch zeros; results are never used.
            wsrc = consts.tile([128, 128], f16)
            wchain = consts.tile([128, 512], f16)
            nc.vector.memset(wsrc[:, :], 0.0)
            nc.vector.memset(wchain[:, :], 0.0)

            def warm(n):
                for i in range(n):
                    wp = pp.tile([128, 512], f32, name="warm", tag="ps")
                    nc.tensor.matmul(wp[:, :], wsrc[:, :], wchain[:, :],
                                     start=True, stop=True)
            warm(12)

            # startup-critical DMAs first, in dependency order of the
            # dequant pipeline: scale table + one-hots, first q tile, then
            # the x slab for the window m-tiles
            wq_r = wq.rearrange("(t p) o -> t p o", p=128)

            xtiles = {}

            def xfetch(j):
                xs = xp.tile([128, kt, 128], f16, name=f"xs{j}", tag="xs")
                nc.sync.dma_start(
                    out=xs, in_=xT[j].rearrange("p (t q) -> p t q", q=128))
                xtiles[j] = xs
                return xs

            qtiles = {}

            def qfetch(t):
                qt = qp.tile([128, O], u8, name=f"qt{t}", tag="qt")
                nc.sync.dma_start(out=qt, in_=wq_r[t])
                qtiles[t] = qt

            s2c_sb = consts.tile([kt * 4, OG], f16)
            nc.sync.dma_start(out=s2c_sb, in_=s2c[:, :])
            qfetch(0)
            ee_r = ee.rearrange("p (t q) -> p t q", q=128)
            e_all = consts.tile([128, kt, 128], f16)
            nc.sync.dma_start(out=e_all[:, 0:4, :], in_=ee_r[:, 0:4, :])
            xfetch(0)
            nc.sync.dma_start(out=e_all[:, 4:, :], in_=ee_r[:, 4:, :])
            xfetch(1)
            # deep qt prefetch: keep many weight-tile DMAs in flight so the
            # dequant pipeline is never paced by a single transfer
            for _t in range(1, 8):
                qfetch(_t)

            # open PSUM accumulation groups for the first WMT m-tiles.
            # (m1, c2) is left out so one PSUM bank stays free: at the window
            # close the steady state can start immediately on it while the
            # bias-adds drain the window banks.
            wgroups = [(mt, ci) for mt in range(WMT)
                       for ci in range(len(chunks))][:-1]
            wps = {}
            for mt, ci in wgroups:
                wps[mt, ci] = pp.tile([128, 512], f32,
                                      name=f"wps{mt}_{ci}", tag="ps")

            # compact one-hot results are packed 11-per-PSUM-bank (43 cols
            # each) so the expansion matmuls never wait on the DVE copies
            # that drain them (a stalled expansion matmul would also block
            # every consume matmul queued behind it).
            sps_banks = {}

            def expand(t):
                # route scale rows 4t..4t+3 onto partitions: compact one-hot
                # matmul [K=128] x [128, 43] -> psum slice, then copy to the
                # resident compact-scale tile.
                if t + 8 < kt:
                    qfetch(t + 8)
                qt = qtiles.pop(t)
                g, off = t // 11, (t % 11) * OG
                if t % 11 == 0:
                    sps_banks[g] = spp.tile([128, 512], f32, name=f"sps{g}",
                                            tag="sps")
                sps = sps_banks[g]
                nc.tensor.matmul(sps[:, off:off + OG], e_all[:, t, :],
                                 s2c_sb[:, :], start=True, stop=True)
                nc.vector.tensor_copy(scomp[:, t, :], sps[:, off:off + OG])
                # dequant: w = (q - 7.5) * scale, scale broadcast 32-wide
                nc.vector.scalar_tensor_tensor(
                    w_tiles[t][:, :], qt[:, :], 7.5,
                    scomp[:, t, :, None].broadcast_to([128, OG, 32]),
                    op0=sub, op1=mult)

            def consume(t):
                for mt, ci in wgroups:
                    o0, on = chunks[ci]
                    nc.tensor.matmul(
                        wps[mt, ci][:, :on],
                        xtiles[mt][:, t, :],
                        w_tiles[t][:, o0:o0 + on],
                        start=(t == 0), stop=(t == kt - 1),
                    )

            bias_sb = consts.tile([128, O], f32)
            LA = 3  # expand() runs LA k-tiles ahead of consume()
            expand(0)
            warm(4)
            expand(1)
            # bias is only needed at the window close; issue its DMA now so
            # it is well clear of the critical startup transfers
            nc.sync.dma_start(out=bias_sb, in_=bb[:, :])
            expand(2)
            warm(4)
            for t in range(LA, kt):
                expand(t)
                consume(t - LA)
                # prefetch the first steady-state x m-tiles mid-window,
                # clear of both the startup transfers and the window close
                if t == 16:
                    xfetch(WMT)
                elif t == 24:
                    xfetch(WMT + 1)
            for t in range(kt - LA, kt):
                consume(t)

            # the left-out group runs on the spare bank right at the window
            # close, keeping the PE busy while the DVE drains the window banks
            lm, lc = WMT - 1, len(chunks) - 1
            lo0, lon = chunks[lc]
            lps = pp.tile([128, 512], f32, name="lps", tag="ps")
            for t in range(kt):
                nc.tensor.matmul(
                    lps[:, :lon],
                    xtiles[lm][:, t, :],
                    w_tiles[t][:, lo0:lo0 + lon],
                    start=(t == 0), stop=(t == kt - 1),
                )

            # close the window groups
            obs = {}
            for mt in range(WMT):
                obs[mt] = op.tile([128, O], f32, name=f"wob{mt}", tag="ob")
            for mt, ci in wgroups:
                o0, on = chunks[ci]
                nc.vector.tensor_add(
                    obs[mt][:, o0:o0 + on], wps[mt, ci][:, :on],
                    bias_sb[:, o0:o0 + on])
                nc.sync.dma_start(
                    out=y[mt * 128:(mt + 1) * 128, o0:o0 + on],
                    in_=obs[mt][:, o0:o0 + on])
            nc.vector.tensor_add(
                obs[lm][:, lo0:lo0 + lon], lps[:, :lon],
                bias_sb[:, lo0:lo0 + lon])
            nc.sync.dma_start(
                out=y[lm * 128:(lm + 1) * 128, lo0:lo0 + lon],
                in_=obs[lm][:, lo0:lo0 + lon])

            # ---- steady state ----
            n_mt = M // 128
            for j in range(WMT, n_mt):
                if j + 2 < n_mt:
                    xfetch(j + 2)
                xs = xtiles.pop(j)
                m0 = j * 128
                ob = op.tile([128, O], f32, name="ob", tag="ob")
                last_mt = j == n_mt - 1
                for ci, (o0, on) in enumerate(chunks):
                    ps = pp.tile([128, 512], f32, tag="ps")
                    for t in range(kt):
                        nc.tensor.matmul(
                            ps[:, :on],
                            xs[:, t, :],
                            w_tiles[t][:, o0:o0 + on],
                            start=(t == 0), stop=(t == kt - 1),
                        )
                    if last_mt and ci == len(chunks) - 1:
                        # split the very last close so the final DMA chain is
                        # short and overlaps the preceding adds
                        step = on // 4
                        for s0 in range(o0, o0 + on, step):
                            nc.vector.tensor_add(
                                ob[:, s0:s0 + step], ps[:, s0 - o0:s0 - o0 + step],
                                bias_sb[:, s0:s0 + step])
                            nc.sync.dma_start(
                                out=y[m0:m0 + 128, s0:s0 + step],
                                in_=ob[:, s0:s0 + step])
                    else:
                        nc.vector.tensor_add(
                            ob[:, o0:o0 + on], ps[:, :on], bias_sb[:, o0:o0 + on])
                        nc.sync.dma_start(
                            out=y[m0:m0 + 128, o0:o0 + on],
                            in_=ob[:, o0:o0 + on])
    nc.compile()
    return nc


def _get_prog(M=None, O=None, kt=None):
    key = (M or 8192, O or O_SH, kt or KT)
    if key not in _PROG:
        _PROG[key] = _build(*key)
    return _PROG[key]


def _in_maps(x, weight_q4, weight_norm, bias, n_cores=N_CORES):
    x = np.asarray(x)
    M = x.size // IN_F
    kt = IN_F // 128
    X = np.asarray(x, np.float32).reshape(M, IN_F).astype(np.float16)
    # (m-tile j, partition p, k-tile t, token q): per-partition-contiguous
    # slabs so each x DMA is 128 x 8KB descriptors
    xT = np.ascontiguousarray(
        X.reshape(M // 128, 128, kt, 128).transpose(0, 3, 2, 1))

    q = np.asarray(weight_q4).astype(np.uint8)          # (O, 128, 16)
    low = q & 15
    high = q >> 4
    w8 = np.stack((low, high), axis=-1).reshape(OUT_F, IN_F)
    wqT = np.ascontiguousarray(w8.T)                    # (4096, 11008) u8

    # compact scales: one column per 32-wide out-feature group
    nf = np.asarray(weight_norm, np.float32)[:, :, 0]   # (344, 128)
    s2cT = np.ascontiguousarray(
        (nf * (2.0 / 15.0)).T).astype(np.float16)       # (128, 344)

    bias = np.asarray(bias, np.float32)

    # one-hot matrices for on-device scale-row routing:
    # E_t[r, p] = 1 iff r == 4t + p//32  ->  (E_t.T @ s)[p, j] = s[4t+p//32, j]
    e_host = np.zeros((128, kt, 128), np.float16)
    p_idx = np.arange(128)
    for t in range(kt):
        e_host[4 * t + p_idx // 32, t, p_idx] = 1.0
    e_host = e_host.reshape(128, kt * 128)

    o_sh = OUT_F // n_cores
    og = o_sh // 32
    maps = []
    for c in range(n_cores):
        sl = slice(c * o_sh, (c + 1) * o_sh)
        maps.append({
            "xT": xT,
            "wq": np.ascontiguousarray(wqT[:, sl]),
            "s2c": np.ascontiguousarray(s2cT[:, c * og:(c + 1) * og]),
            "ee": e_host,
            "bb": np.ascontiguousarray(
                np.broadcast_to(bias[sl], (128, o_sh))),
        })
    return maps


def kernel(x, weight_q4, weight_norm, bias):
    from concourse.bass_utils import run_bass_kernel_spmd
    x = np.asarray(x)
    maps = _in_maps(x, weight_q4, weight_norm, bias)
    nc = _get_prog(M=x.size // IN_F)
    res = run_bass_kernel_spmd(nc, maps, core_ids=list(range(N_CORES)))
    out = np.concatenate([r["y"] for r in res.results], axis=1)
    return out.reshape(x.shape[0], x.shape[1], OUT_F)


# revision 21
# speedup vs baseline: 1.0777x; 1.0777x over previous
"""Bass/Trainium2 kernel for LinearRowShared4Bit.

y[b,s,o] = sum_i x[b,s,i] * W[o,i] + bias[o]
W[o,i]   = (2*q[o,i]/15 - 1) * norm[o//32, i//32]   (q = 4-bit nibbles)

Sharding: out_features (11008) split 1376-per-core across 8 cores; x replicated.

Host does layout only (transpose / nibble->byte unpack / norm regrouping);
all dequant arithmetic ((q-7.5)*(2*norm/15)) and the matmul run on device.

Device per core (v3 schedule):
  - the per-(k-block, o-group) scales have only 43 distinct columns per core
    (scales are shared by 32 consecutive out features), so the one-hot
    expansion matmul that routes scale rows onto the 128 partitions only
    produces a compact [128, 43] tile per k-tile (~150ns each); the DVE
    dequant reads it through a stride-0 broadcast access pattern,
  - W is dequantized into a resident SBUF tile [128, 32, 1376] fp16
    (11.3 MB) k-tile by k-tile, with per-tile DMAs ordered by criticality,
  - dequant overlaps main compute: the first two m-tiles' PSUM accumulation
    groups are held open across the dequant loop and consume each W k-tile
    as soon as it is ready,
  - the remaining 128-token tiles run the usual 3 o-chunk x 32 k-tile
    accumulation, bias-add on DVE, fp32 rows DMA'd out.
"""

import numpy as np

IN_F = 4096
OUT_F = 11008
N_CORES = 8
O_SH = OUT_F // N_CORES  # 1376
KT = IN_F // 128         # 32 k-tiles
MS = 256                 # tokens per x-slab DMA
WMT = 2                  # m-tiles accumulated during the dequant window

_PROG = {}


def _build(M, O, kt):
    import concourse.mybir as mybir
    import concourse.tile as tile
    from concourse import bacc

    f16, f32, u8 = mybir.dt.float16, mybir.dt.float32, mybir.dt.uint8
    nc = bacc.Bacc("TRN2", target_bir_lowering=False, debug=False,
                   num_devices=N_CORES)
    K = kt * 128
    OG = O // 32  # distinct scale columns (43)
    xT = nc.dram_tensor("xT", (M // 128, 128, K), f16, kind="ExternalInput")
    wq = nc.dram_tensor("wq", (K, O), u8, kind="ExternalInput")
    s2c = nc.dram_tensor("s2c", (kt * 4, OG), f16, kind="ExternalInput")
    ee = nc.dram_tensor("ee", (128, kt * 128), f16, kind="ExternalInput")
    bb = nc.dram_tensor("bb", (128, O), f32, kind="ExternalInput")
    y = nc.dram_tensor("y", (M, O), f32, kind="ExternalOutput")

    chunks = [(o0, min(512, O - o0)) for o0 in range(0, O, 512)]
    ms_cnt = M // MS
    sub, mult = mybir.AluOpType.subtract, mybir.AluOpType.mult

    with tile.TileContext(nc) as tc:
        with (
            tc.tile_pool(name="wres", bufs=32) as wres,
            tc.tile_pool(name="consts", bufs=1) as consts,
            tc.tile_pool(name="qp", bufs=16) as qp,
            tc.tile_pool(name="sxp", bufs=4) as sxp,
            tc.tile_pool(name="xp", bufs=4) as xp,
            tc.tile_pool(name="op", bufs=3) as op,
            tc.tile_pool(name="pp", bufs=6, space="PSUM") as pp,
            tc.tile_pool(name="spp", bufs=2, space="PSUM") as spp,
        ):
            # one W tile per k-tile: exact per-tile dependencies, so a
            # consume matmul only waits on its own k-tile's dequant
            w_tiles = [wres.tile([128, O], f16, name=f"w{t}", tag="w")
                       for t in range(kt)]
            scomp = consts.tile([128, kt, OG], f16)

            # PE warm-up: the tensor engine's clock gate (HAM) keeps the PE
            # at half clock until it has seen ~3.4us of *sustained* activity,
            # and re-throttles after ~3.4us of idle. The startup DMAs take
            # ~15us, so burn a solid burst of back-to-back dummy matmuls
            # first; by the time the real matmuls start the PE is at full
            # clock and the idle gap to them is under the re-throttle window.
            # Reads scratch zeros; results are never used.
            wsrc = consts.tile([128, 128], f16)
            wchain = consts.tile([128, 512], f16)
            nc.vector.memset(wsrc[:, :], 0.0)
            nc.vector.memset(wchain[:, :], 0.0)

            def warm(n):
                for i in range(n):
                    wp = pp.tile([128, 512], f32, name="warm", tag="ps")
                    nc.tensor.matmul(wp[:, :], wsrc[:, :], wchain[:, :],
                                     start=True, stop=True)
            warm(12)

            # startup-critical DMAs first, in dependency order of the
            # dequant pipeline: scale table + one-hots, first q tile, then
            # the x slab for the window m-tiles
            wq_r = wq.rearrange("(t p) o -> t p o", p=128)

            xtiles = {}

            def xfetch(j):
                xs = xp.tile([128, kt, 128], f16, name=f"xs{j}", tag="xs")
                nc.sync.dma_start(
                    out=xs, in_=xT[j].rearrange("p (t q) -> p t q", q=128))
                xtiles[j] = xs
                return xs

            qtiles = {}

            def qfetch(t):
                qt = qp.tile([128, O], u8, name=f"qt{t}", tag="qt")
                nc.sync.dma_start(out=qt, in_=wq_r[t])
                qtiles[t] = qt

            s2c_sb = consts.tile([kt * 4, OG], f16)
            nc.sync.dma_start(out=s2c_sb, in_=s2c[:, :])
            qfetch(0)
            ee_r = ee.rearrange("p (t q) -> p t q", q=128)
            e_all = consts.tile([128, kt, 128], f16)
            nc.sync.dma_start(out=e_all[:, 0:4, :], in_=ee_r[:, 0:4, :])
            xfetch(0)
            nc.sync.dma_start(out=e_all[:, 4:, :], in_=ee_r[:, 4:, :])
            xfetch(1)
            # deep qt prefetch: keep many weight-tile DMAs in flight so the
            # dequant pipeline is never paced by a single transfer
            for _t in range(1, 8):
                qfetch(_t)

            # open PSUM accumulation groups for the first WMT m-tiles.
            # (m1, c2) is left out so one PSUM bank stays free: at the window
            # close the steady state can start immediately on it while the
            # bias-adds drain the window banks.
            wgroups = [(mt, ci) for mt in range(WMT)
                       for ci in range(len(chunks))][:-1]
            wps = {}
            for mt, ci in wgroups:
                wps[mt, ci] = pp.tile([128, 512], f32,
                                      name=f"wps{mt}_{ci}", tag="ps")

            # compact one-hot results are packed 11-per-PSUM-bank (43 cols
            # each) so the expansion matmuls never wait on the DVE copies
            # that drain them (a stalled expansion matmul would also block
            # every consume matmul queued behind it).
            sps_banks = {}

            def expand(t):
                # route scale rows 4t..4t+3 onto partitions: compact one-hot
                # matmul [K=128] x [128, 43] -> psum slice, then copy to the
                # resident compact-scale tile.
                if t + 8 < kt:
                    qfetch(t + 8)
                qt = qtiles.pop(t)
                g, off = t // 11, (t % 11) * OG
                if t % 11 == 0:
                    sps_banks[g] = spp.tile([128, 512], f32, name=f"sps{g}",
                                            tag="sps")
                sps = sps_banks[g]
                nc.tensor.matmul(sps[:, off:off + OG], e_all[:, t, :],
                                 s2c_sb[:, :], start=True, stop=True)
                nc.vector.tensor_copy(scomp[:, t, :], sps[:, off:off + OG])
                # expand the 43 compact scales to full width on the otherwise
                # idle GPSIMD engine: a stride-0 operand halves DVE
                # throughput, so keep the broadcast off the DVE's critical
                # dequant op
                sx = sxp.tile([128, O], f16, name="sx", tag="sx")
                nc.gpsimd.tensor_copy(
                    sx[:, :],
                    scomp[:, t, :, None].broadcast_to([128, OG, 32]))
                # dequant: w = (q - 7.5) * scale, all operands contiguous
                nc.vector.scalar_tensor_tensor(
                    w_tiles[t][:, :], qt[:, :], 7.5, sx[:, :],
                    op0=sub, op1=mult)

            def consume(t):
                for mt, ci in wgroups:
                    o0, on = chunks[ci]
                    nc.tensor.matmul(
                        wps[mt, ci][:, :on],
                        xtiles[mt][:, t, :],
                        w_tiles[t][:, o0:o0 + on],
                        start=(t == 0), stop=(t == kt - 1),
                    )

            bias_sb = consts.tile([128, O], f32)
            LA = 3  # expand() runs LA k-tiles ahead of consume()
            expand(0)
            warm(4)
            expand(1)
            # bias is only needed at the window close; issue its DMA now so
            # it is well clear of the critical startup transfers
            nc.sync.dma_start(out=bias_sb, in_=bb[:, :])
            expand(2)
            warm(4)
            for t in range(LA, kt):
                expand(t)
                consume(t - LA)
                # prefetch the first steady-state x m-tiles mid-window,
                # clear of both the startup transfers and the window close
                if t == 16:
                    xfetch(WMT)
                elif t == 24:
                    xfetch(WMT + 1)
            for t in range(kt - LA, kt):
                consume(t)

            # the left-out group runs on the spare bank right at the window
            # close, keeping the PE busy while the DVE drains the window banks
            lm, lc = WMT - 1, len(chunks) - 1
            lo0, lon = chunks[lc]
            lps = pp.tile([128, 512], f32, name="lps", tag="ps")
            for t in range(kt):
                nc.tensor.matmul(
                    lps[:, :lon],
                    xtiles[lm][:, t, :],
                    w_tiles[t][:, lo0:lo0 + lon],
                    start=(t == 0), stop=(t == kt - 1),
                )

            # close the window groups
            obs = {}
            for mt in range(WMT):
                obs[mt] = op.tile([128, O], f32, name=f"wob{mt}", tag="ob")
            for mt, ci in wgroups:
                o0, on = chunks[ci]
                nc.vector.tensor_add(
                    obs[mt][:, o0:o0 + on], wps[mt, ci][:, :on],
                    bias_sb[:, o0:o0 + on])
                nc.sync.dma_start(
                    out=y[mt * 128:(mt + 1) * 128, o0:o0 + on],
                    in_=obs[mt][:, o0:o0 + on])
            nc.vector.tensor_add(
                obs[lm][:, lo0:lo0 + lon], lps[:, :lon],
                bias_sb[:, lo0:lo0 + lon])
            nc.sync.dma_start(
                out=y[lm * 128:(lm + 1) * 128, lo0:lo0 + lon],
                in_=obs[lm][:, lo0:lo0 + lon])

            # ---- steady state ----
            n_mt = M // 128
            for j in range(WMT, n_mt):
                if j + 2 < n_mt:
                    xfetch(j + 2)
                xs = xtiles.pop(j)
                m0 = j * 128
                ob = op.tile([128, O], f32, name="ob", tag="ob")
                last_mt = j == n_mt - 1
                for ci, (o0, on) in enumerate(chunks):
                    ps = pp.tile([128, 512], f32, tag="ps")
                    for t in range(kt):
                        nc.tensor.matmul(
                            ps[:, :on],
                            xs[:, t, :],
                            w_tiles[t][:, o0:o0 + on],
                            start=(t == 0), stop=(t == kt - 1),
                        )
                    if last_mt and ci == len(chunks) - 1:
                        # split the very last close so the final DMA chain is
                        # short and overlaps the preceding adds
                        step = on // 4
                        for s0 in range(o0, o0 + on, step):
                            nc.vector.tensor_add(
                                ob[:, s0:s0 + step], ps[:, s0 - o0:s0 - o0 + step],
                                bias_sb[:, s0:s0 + step])
                            nc.sync.dma_start(
                                out=y[m0:m0 + 128, s0:s0 + step],
                                in_=ob[:, s0:s0 + step])
                    else:
                        nc.vector.tensor_add(
                            ob[:, o0:o0 + on], ps[:, :on], bias_sb[:, o0:o0 + on])
                        nc.sync.dma_start(
                            out=y[m0:m0 + 128, o0:o0 + on],
                            in_=ob[:, o0:o0 + on])
    nc.compile()
    return nc


def _get_prog(M=None, O=None, kt=None):
    key = (M or 8192, O or O_SH, kt or KT)
    if key not in _PROG:
        _PROG[key] = _build(*key)
    return _PROG[key]


def _in_maps(x, weight_q4, weight_norm, bias, n_cores=N_CORES):
    x = np.asarray(x)
    M = x.size // IN_F
    kt = IN_F // 128
    X = np.asarray(x, np.float32).reshape(M, IN_F).astype(np.float16)
    # (m-tile j, partition p, k-tile t, token q): per-partition-contiguous
    # slabs so each x DMA is 128 x 8KB descriptors
    xT = np.ascontiguousarray(
        X.reshape(M // 128, 128, kt, 128).transpose(0, 3, 2, 1))

    q = np.asarray(weight_q4).astype(np.uint8)          # (O, 128, 16)
    low = q & 15
    high = q >> 4
    w8 = np.stack((low, high), axis=-1).reshape(OUT_F, IN_F)
    wqT = np.ascontiguousarray(w8.T)                    # (4096, 11008) u8

    # compact scales: one column per 32-wide out-feature group
    nf = np.asarray(weight_norm, np.float32)[:, :, 0]   # (344, 128)
    s2cT = np.ascontiguousarray(
        (nf * (2.0 / 15.0)).T).astype(np.float16)       # (128, 344)

    bias = np.asarray(bias, np.float32)

    # one-hot matrices for on-device scale-row routing:
    # E_t[r, p] = 1 iff r == 4t + p//32  ->  (E_t.T @ s)[p, j] = s[4t+p//32, j]
    e_host = np.zeros((128, kt, 128), np.float16)
    p_idx = np.arange(128)
    for t in range(kt):
        e_host[4 * t + p_idx // 32, t, p_idx] = 1.0
    e_host = e_host.reshape(128, kt * 128)

    o_sh = OUT_F // n_cores
    og = o_sh // 32
    maps = []
    for c in range(n_cores):
        sl = slice(c * o_sh, (c + 1) * o_sh)
        maps.append({
            "xT": xT,
            "wq": np.ascontiguousarray(wqT[:, sl]),
            "s2c": np.ascontiguousarray(s2cT[:, c * og:(c + 1) * og]),
            "ee": e_host,
            "bb": np.ascontiguousarray(
                np.broadcast_to(bias[sl], (128, o_sh))),
        })
    return maps


def kernel(x, weight_q4, weight_norm, bias):
    from concourse.bass_utils import run_bass_kernel_spmd
    x = np.asarray(x)
    maps = _in_maps(x, weight_q4, weight_norm, bias)
    nc = _get_prog(M=x.size // IN_F)
    res = run_bass_kernel_spmd(nc, maps, core_ids=list(range(N_CORES)))
    out = np.concatenate([r["y"] for r in res.results], axis=1)
    return out.reshape(x.shape[0], x.shape[1], OUT_F)


# revision 22
# speedup vs baseline: 1.1972x; 1.1109x over previous
"""Bass/Trainium2 kernel for LinearRowShared4Bit.

y[b,s,o] = sum_i x[b,s,i] * W[o,i] + bias[o]
W[o,i]   = (2*q[o,i]/15 - 1) * norm[o//32, i//32]   (q = 4-bit nibbles)

Sharding: out_features (11008) split 1376-per-core across 8 cores; x replicated.

Host does layout only (transpose / nibble->byte unpack / norm regrouping);
all dequant arithmetic ((q-7.5)*(2*norm/15)) and the matmul run on device.

Device per core (v3 schedule):
  - the per-(k-block, o-group) scales have only 43 distinct columns per core
    (scales are shared by 32 consecutive out features), so the one-hot
    expansion matmul that routes scale rows onto the 128 partitions only
    produces a compact [128, 43] tile per k-tile (~150ns each); the DVE
    dequant reads it through a stride-0 broadcast access pattern,
  - W is dequantized into a resident SBUF tile [128, 32, 1376] fp16
    (11.3 MB) k-tile by k-tile, with per-tile DMAs ordered by criticality,
  - dequant overlaps main compute: the first two m-tiles' PSUM accumulation
    groups are held open across the dequant loop and consume each W k-tile
    as soon as it is ready,
  - the remaining 128-token tiles run the usual 3 o-chunk x 32 k-tile
    accumulation, bias-add on DVE, fp32 rows DMA'd out.
"""

import numpy as np

IN_F = 4096
OUT_F = 11008
N_CORES = 8
O_SH = OUT_F // N_CORES  # 1376
KT = IN_F // 128         # 32 k-tiles
MS = 256                 # tokens per x-slab DMA
WMT = 2                  # m-tiles accumulated during the dequant window

_PROG = {}


def _build(M, O, kt):
    import concourse.mybir as mybir
    import concourse.tile as tile
    from concourse import bacc

    f16, f32, u8 = mybir.dt.float16, mybir.dt.float32, mybir.dt.uint8
    nc = bacc.Bacc("TRN2", target_bir_lowering=False, debug=False,
                   num_devices=N_CORES)
    K = kt * 128
    OG = O // 32  # distinct scale columns (43)
    xT = nc.dram_tensor("xT", (M // 128, 128, K), f16, kind="ExternalInput")
    wq = nc.dram_tensor("wq", (K, O), u8, kind="ExternalInput")
    s2c = nc.dram_tensor("s2c", (kt * 4, OG), f16, kind="ExternalInput")
    ee = nc.dram_tensor("ee", (128, kt * 128), f16, kind="ExternalInput")
    bb = nc.dram_tensor("bb", (128, O), f32, kind="ExternalInput")
    y = nc.dram_tensor("y", (M, O), f32, kind="ExternalOutput")

    chunks = [(o0, min(512, O - o0)) for o0 in range(0, O, 512)]
    ms_cnt = M // MS
    sub, mult = mybir.AluOpType.subtract, mybir.AluOpType.mult

    with tile.TileContext(nc) as tc:
        with (
            tc.tile_pool(name="wres", bufs=32) as wres,
            tc.tile_pool(name="consts", bufs=1) as consts,
            tc.tile_pool(name="qp", bufs=16) as qp,
            tc.tile_pool(name="xp", bufs=4) as xp,
            tc.tile_pool(name="op", bufs=3) as op,
            tc.tile_pool(name="pp", bufs=6, space="PSUM") as pp,
            tc.tile_pool(name="spp", bufs=2, space="PSUM") as spp,
        ):
            # one W tile per k-tile: exact per-tile dependencies, so a
            # consume matmul only waits on its own k-tile's dequant
            w_tiles = [wres.tile([128, O], f16, name=f"w{t}", tag="w")
                       for t in range(kt)]
            scomp = consts.tile([128, kt, OG], f16)

            # PE warm-up: the tensor engine's clock gate (HAM) keeps the PE
            # at half clock until it has seen ~3.4us of *sustained* activity,
            # and re-throttles after ~3.4us of idle. The startup DMAs take
            # ~15us, so burn a solid burst of back-to-back dummy matmuls
            # first; by the time the real matmuls start the PE is at full
            # clock and the idle gap to them is under the re-throttle window.
            # Reads scratch zeros; results are never used.
            wsrc = consts.tile([128, 128], f16)
            wchain = consts.tile([128, 512], f16)
            nc.vector.memset(wsrc[:, :], 0.0)
            nc.vector.memset(wchain[:, :], 0.0)

            def warm(n):
                for i in range(n):
                    wp = pp.tile([128, 512], f32, name="warm", tag="ps")
                    nc.tensor.matmul(wp[:, :], wsrc[:, :], wchain[:, :],
                                     start=True, stop=True)
            warm(12)

            # startup-critical DMAs first, in dependency order of the
            # dequant pipeline: scale table + one-hots, first q tile, then
            # the x slab for the window m-tiles
            wq_r = wq.rearrange("(t p) o -> t p o", p=128)

            xtiles = {}

            def xfetch(j):
                xs = xp.tile([128, kt, 128], f16, name=f"xs{j}", tag="xs")
                nc.sync.dma_start(
                    out=xs, in_=xT[j].rearrange("p (t q) -> p t q", q=128))
                xtiles[j] = xs
                return xs

            qtiles = {}

            def qfetch(t):
                qt = qp.tile([128, O], u8, name=f"qt{t}", tag="qt")
                nc.sync.dma_start(out=qt, in_=wq_r[t])
                qtiles[t] = qt

            s2c_sb = consts.tile([kt * 4, OG], f16)
            nc.sync.dma_start(out=s2c_sb, in_=s2c[:, :])
            qfetch(0)
            ee_r = ee.rearrange("p (t q) -> p t q", q=128)
            e_all = consts.tile([128, kt, 128], f16)
            nc.sync.dma_start(out=e_all[:, 0:4, :], in_=ee_r[:, 0:4, :])
            xfetch(0)
            nc.sync.dma_start(out=e_all[:, 4:, :], in_=ee_r[:, 4:, :])
            xfetch(1)
            # deep qt prefetch: keep many weight-tile DMAs in flight so the
            # dequant pipeline is never paced by a single transfer
            for _t in range(1, 8):
                qfetch(_t)

            # open PSUM accumulation groups for the first WMT m-tiles.
            # (m1, c2) is left out so one PSUM bank stays free: at the window
            # close the steady state can start immediately on it while the
            # bias-adds drain the window banks.
            wgroups = [(mt, ci) for mt in range(WMT)
                       for ci in range(len(chunks))][:-1]
            wps = {}
            for mt, ci in wgroups:
                wps[mt, ci] = pp.tile([128, 512], f32,
                                      name=f"wps{mt}_{ci}", tag="ps")

            # compact one-hot results are packed 11-per-PSUM-bank (43 cols
            # each) so the expansion matmuls never wait on the DVE copies
            # that drain them (a stalled expansion matmul would also block
            # every consume matmul queued behind it).
            sps_banks = {}

            def expand(t):
                # route scale rows 4t..4t+3 onto partitions: compact one-hot
                # matmul [K=128] x [128, 43] -> psum slice, then copy to the
                # resident compact-scale tile.
                if t + 8 < kt:
                    qfetch(t + 8)
                qt = qtiles.pop(t)
                g, off = t // 11, (t % 11) * OG
                if t % 11 == 0:
                    sps_banks[g] = spp.tile([128, 512], f32, name=f"sps{g}",
                                            tag="sps")
                sps = sps_banks[g]
                nc.tensor.matmul(sps[:, off:off + OG], e_all[:, t, :],
                                 s2c_sb[:, :], start=True, stop=True)
                nc.vector.tensor_copy(scomp[:, t, :], sps[:, off:off + OG])
                # dequant: w = (q - 7.5) * scale, scale broadcast 32-wide
                nc.vector.scalar_tensor_tensor(
                    w_tiles[t][:, :], qt[:, :], 7.5,
                    scomp[:, t, :, None].broadcast_to([128, OG, 32]),
                    op0=sub, op1=mult)

            def consume(t):
                for mt, ci in wgroups:
                    o0, on = chunks[ci]
                    nc.tensor.matmul(
                        wps[mt, ci][:, :on],
                        xtiles[mt][:, t, :],
                        w_tiles[t][:, o0:o0 + on],
                        start=(t == 0), stop=(t == kt - 1),
                    )

            bias_sb = consts.tile([128, O], f32)
            LA = 3  # expand() runs LA k-tiles ahead of consume()
            expand(0)
            warm(4)
            expand(1)
            # bias is only needed at the window close; issue its DMA now so
            # it is well clear of the critical startup transfers
            nc.sync.dma_start(out=bias_sb, in_=bb[:, :])
            expand(2)
            warm(4)
            for t in range(LA, kt):
                expand(t)
                consume(t - LA)
                # prefetch the first steady-state x m-tiles mid-window,
                # clear of both the startup transfers and the window close
                if t == 16:
                    xfetch(WMT)
                elif t == 24:
                    xfetch(WMT + 1)
            for t in range(kt - LA, kt):
                consume(t)

            # the left-out group runs on the spare bank right at the window
            # close, keeping the PE busy while the DVE drains the window banks
            lm, lc = WMT - 1, len(chunks) - 1
            lo0, lon = chunks[lc]
            lps = pp.tile([128, 512], f32, name="lps", tag="ps")
            for t in range(kt):
                nc.tensor.matmul(
                    lps[:, :lon],
                    xtiles[lm][:, t, :],
                    w_tiles[t][:, lo0:lo0 + lon],
                    start=(t == 0), stop=(t == kt - 1),
                )

            # close the window groups
            obs = {}
            for mt in range(WMT):
                obs[mt] = op.tile([128, O], f32, name=f"wob{mt}", tag="ob")
            for mt, ci in wgroups:
                o0, on = chunks[ci]
                nc.vector.tensor_add(
                    obs[mt][:, o0:o0 + on], wps[mt, ci][:, :on],
                    bias_sb[:, o0:o0 + on])
                nc.sync.dma_start(
                    out=y[mt * 128:(mt + 1) * 128, o0:o0 + on],
                    in_=obs[mt][:, o0:o0 + on])
            nc.vector.tensor_add(
                obs[lm][:, lo0:lo0 + lon], lps[:, :lon],
                bias_sb[:, lo0:lo0 + lon])
            nc.sync.dma_start(
                out=y[lm * 128:(lm + 1) * 128, lo0:lo0 + lon],
                in_=obs[lm][:, lo0:lo0 + lon])

            # ---- steady state ----
            n_mt = M // 128
            for j in range(WMT, n_mt):
                if j + 2 < n_mt:
                    xfetch(j + 2)
                xs = xtiles.pop(j)
                m0 = j * 128
                ob = op.tile([128, O], f32, name="ob", tag="ob")
                last_mt = j == n_mt - 1
                for ci, (o0, on) in enumerate(chunks):
                    ps = pp.tile([128, 512], f32, tag="ps")
                    for t in range(kt):
                        nc.tensor.matmul(
                            ps[:, :on],
                            xs[:, t, :],
                            w_tiles[t][:, o0:o0 + on],
                            start=(t == 0), stop=(t == kt - 1),
                        )
                    if last_mt and ci == len(chunks) - 1:
                        # split the very last close so the final DMA chain is
                        # short and overlaps the preceding adds
                        step = on // 4
                        for s0 in range(o0, o0 + on, step):
                            nc.vector.tensor_add(
                                ob[:, s0:s0 + step], ps[:, s0 - o0:s0 - o0 + step],
                                bias_sb[:, s0:s0 + step])
                            nc.sync.dma_start(
                                out=y[m0:m0 + 128, s0:s0 + step],
                                in_=ob[:, s0:s0 + step])
                    else:
                        nc.vector.tensor_add(
                            ob[:, o0:o0 + on], ps[:, :on], bias_sb[:, o0:o0 + on])
                        nc.sync.dma_start(
                            out=y[m0:m0 + 128, o0:o0 + on],
                            in_=ob[:, o0:o0 + on])
    nc.compile()
    return nc


def _get_prog(M=None, O=None, kt=None):
    key = (M or 8192, O or O_SH, kt or KT)
    if key not in _PROG:
        _PROG[key] = _build(*key)
    return _PROG[key]


def _in_maps(x, weight_q4, weight_norm, bias, n_cores=N_CORES):
    x = np.asarray(x)
    M = x.size // IN_F
    kt = IN_F // 128
    X = np.asarray(x, np.float32).reshape(M, IN_F).astype(np.float16)
    # (m-tile j, partition p, k-tile t, token q): per-partition-contiguous
    # slabs so each x DMA is 128 x 8KB descriptors
    xT = np.ascontiguousarray(
        X.reshape(M // 128, 128, kt, 128).transpose(0, 3, 2, 1))

    q = np.asarray(weight_q4).astype(np.uint8)          # (O, 128, 16)
    low = q & 15
    high = q >> 4
    w8 = np.stack((low, high), axis=-1).reshape(OUT_F, IN_F)
    wqT = np.ascontiguousarray(w8.T)                    # (4096, 11008) u8

    # compact scales: one column per 32-wide out-feature group
    nf = np.asarray(weight_norm, np.float32)[:, :, 0]   # (344, 128)
    s2cT = np.ascontiguousarray(
        (nf * (2.0 / 15.0)).T).astype(np.float16)       # (128, 344)

    bias = np.asarray(bias, np.float32)

    # one-hot matrices for on-device scale-row routing:
    # E_t[r, p] = 1 iff r == 4t + p//32  ->  (E_t.T @ s)[p, j] = s[4t+p//32, j]
    e_host = np.zeros((128, kt, 128), np.float16)
    p_idx = np.arange(128)
    for t in range(kt):
        e_host[4 * t + p_idx // 32, t, p_idx] = 1.0
    e_host = e_host.reshape(128, kt * 128)

    o_sh = OUT_F // n_cores
    og = o_sh // 32
    maps = []
    for c in range(n_cores):
        sl = slice(c * o_sh, (c + 1) * o_sh)
        maps.append({
            "xT": xT,
            "wq": np.ascontiguousarray(wqT[:, sl]),
            "s2c": np.ascontiguousarray(s2cT[:, c * og:(c + 1) * og]),
            "ee": e_host,
            "bb": np.ascontiguousarray(
                np.broadcast_to(bias[sl], (128, o_sh))),
        })
    return maps


def kernel(x, weight_q4, weight_norm, bias):
    from concourse.bass_utils import run_bass_kernel_spmd
    x = np.asarray(x)
    maps = _in_maps(x, weight_q4, weight_norm, bias)
    nc = _get_prog(M=x.size // IN_F)
    res = run_bass_kernel_spmd(nc, maps, core_ids=list(range(N_CORES)))
    out = np.concatenate([r["y"] for r in res.results], axis=1)
    return out.reshape(x.shape[0], x.shape[1], OUT_F)


# revision 24
# speedup vs baseline: 1.1977x; 1.0004x over previous
"""Bass/Trainium2 kernel for LinearRowShared4Bit.

y[b,s,o] = sum_i x[b,s,i] * W[o,i] + bias[o]
W[o,i]   = (2*q[o,i]/15 - 1) * norm[o//32, i//32]   (q = 4-bit nibbles)

Sharding: out_features (11008) split 1376-per-core across 8 cores; x replicated.

Host does layout only (transpose / nibble->byte unpack / norm regrouping);
all dequant arithmetic ((q-7.5)*(2*norm/15)) and the matmul run on device.

Device per core (v3 schedule):
  - the per-(k-block, o-group) scales have only 43 distinct columns per core
    (scales are shared by 32 consecutive out features), so the one-hot
    expansion matmul that routes scale rows onto the 128 partitions only
    produces a compact [128, 43] tile per k-tile (~150ns each); the DVE
    dequant reads it through a stride-0 broadcast access pattern,
  - W is dequantized into a resident SBUF tile [128, 32, 1376] fp16
    (11.3 MB) k-tile by k-tile, with per-tile DMAs ordered by criticality,
  - dequant overlaps main compute: the first two m-tiles' PSUM accumulation
    groups are held open across the dequant loop and consume each W k-tile
    as soon as it is ready,
  - the remaining 128-token tiles run the usual 3 o-chunk x 32 k-tile
    accumulation, bias-add on DVE, fp32 rows DMA'd out.
"""

import numpy as np

IN_F = 4096
OUT_F = 11008
N_CORES = 8
O_SH = OUT_F // N_CORES  # 1376
KT = IN_F // 128         # 32 k-tiles
MS = 256                 # tokens per x-slab DMA
WMT = 2                  # m-tiles accumulated during the dequant window

_PROG = {}


def _build(M, O, kt):
    import concourse.mybir as mybir
    import concourse.tile as tile
    from concourse import bacc

    f16, f32, u8 = mybir.dt.float16, mybir.dt.float32, mybir.dt.uint8
    nc = bacc.Bacc("TRN2", target_bir_lowering=False, debug=False,
                   num_devices=N_CORES)
    K = kt * 128
    OG = O // 32  # distinct scale columns (43)
    xT = nc.dram_tensor("xT", (M // 128, 128, K), f16, kind="ExternalInput")
    wq = nc.dram_tensor("wq", (K, O), u8, kind="ExternalInput")
    s2c = nc.dram_tensor("s2c", (kt * 4, OG), f16, kind="ExternalInput")
    ee = nc.dram_tensor("ee", (128, kt * 128), f16, kind="ExternalInput")
    bb = nc.dram_tensor("bb", (128, O), f32, kind="ExternalInput")
    y = nc.dram_tensor("y", (M, O), f32, kind="ExternalOutput")

    chunks = [(o0, min(512, O - o0)) for o0 in range(0, O, 512)]
    ms_cnt = M // MS
    sub, mult = mybir.AluOpType.subtract, mybir.AluOpType.mult

    with tile.TileContext(nc) as tc:
        with (
            tc.tile_pool(name="wres", bufs=32) as wres,
            tc.tile_pool(name="consts", bufs=1) as consts,
            tc.tile_pool(name="qp", bufs=16) as qp,
            tc.tile_pool(name="xp", bufs=4) as xp,
            tc.tile_pool(name="op", bufs=3) as op,
            tc.tile_pool(name="pp", bufs=6, space="PSUM") as pp,
            tc.tile_pool(name="spp", bufs=2, space="PSUM") as spp,
        ):
            # one W tile per k-tile: exact per-tile dependencies, so a
            # consume matmul only waits on its own k-tile's dequant
            w_tiles = [wres.tile([128, O], f16, name=f"w{t}", tag="w")
                       for t in range(kt)]
            scomp = consts.tile([128, kt, OG], f16)

            # PE warm-up: the tensor engine's clock gate (HAM) keeps the PE
            # at half clock until it has seen ~3.4us of *sustained* activity,
            # and re-throttles after ~3.4us of idle. The startup DMAs take
            # ~15us, so burn a solid burst of back-to-back dummy matmuls
            # first; by the time the real matmuls start the PE is at full
            # clock and the idle gap to them is under the re-throttle window.
            # Reads scratch zeros; results are never used.
            wsrc = consts.tile([128, 128], f16)
            wchain = consts.tile([128, 512], f16)
            nc.vector.memset(wsrc[:, :], 0.0)
            nc.vector.memset(wchain[:, :], 0.0)

            def warm(n):
                for i in range(n):
                    wp = pp.tile([128, 512], f32, name="warm", tag="ps")
                    nc.tensor.matmul(wp[:, :], wsrc[:, :], wchain[:, :],
                                     start=True, stop=True)
            warm(12)

            # startup-critical DMAs first, in dependency order of the
            # dequant pipeline: scale table + one-hots, first q tile, then
            # the x slab for the window m-tiles
            wq_r = wq.rearrange("(t p) o -> t p o", p=128)

            xtiles = {}

            def xfetch(j):
                xs = xp.tile([128, kt, 128], f16, name=f"xs{j}", tag="xs")
                nc.sync.dma_start(
                    out=xs, in_=xT[j].rearrange("p (t q) -> p t q", q=128))
                xtiles[j] = xs
                return xs

            qtiles = {}

            def qfetch(t):
                qt = qp.tile([128, O], u8, name=f"qt{t}", tag="qt")
                nc.sync.dma_start(out=qt, in_=wq_r[t])
                qtiles[t] = qt

            s2c_sb = consts.tile([kt * 4, OG], f16)
            nc.sync.dma_start(out=s2c_sb, in_=s2c[:, :])
            qfetch(0)
            ee_r = ee.rearrange("p (t q) -> p t q", q=128)
            e_all = consts.tile([128, kt, 128], f16)
            nc.sync.dma_start(out=e_all[:, 0:4, :], in_=ee_r[:, 0:4, :])
            xfetch(0)
            nc.sync.dma_start(out=e_all[:, 4:, :], in_=ee_r[:, 4:, :])
            xfetch(1)
            # deep qt prefetch: keep many weight-tile DMAs in flight so the
            # dequant pipeline is never paced by a single transfer
            for _t in range(1, 8):
                qfetch(_t)

            # open PSUM accumulation groups for the first WMT m-tiles.
            # (m1, c2) is left out so one PSUM bank stays free: at the window
            # close the steady state can start immediately on it while the
            # bias-adds drain the window banks.
            wgroups = [(mt, ci) for mt in range(WMT)
                       for ci in range(len(chunks))][:-1]
            wps = {}
            for mt, ci in wgroups:
                wps[mt, ci] = pp.tile([128, 512], f32,
                                      name=f"wps{mt}_{ci}", tag="ps")

            # compact one-hot results are packed 11-per-PSUM-bank (43 cols
            # each) so the expansion matmuls never wait on the DVE copies
            # that drain them (a stalled expansion matmul would also block
            # every consume matmul queued behind it).
            sps_banks = {}

            def expand(t):
                # route scale rows 4t..4t+3 onto partitions: compact one-hot
                # matmul [K=128] x [128, 43] -> psum slice, then copy to the
                # resident compact-scale tile.
                if t + 8 < kt:
                    qfetch(t + 8)
                qt = qtiles.pop(t)
                g, off = t // 11, (t % 11) * OG
                if t % 11 == 0:
                    sps_banks[g] = spp.tile([128, 512], f32, name=f"sps{g}",
                                            tag="sps")
                sps = sps_banks[g]
                nc.tensor.matmul(sps[:, off:off + OG], e_all[:, t, :],
                                 s2c_sb[:, :], start=True, stop=True)
                nc.vector.tensor_copy(scomp[:, t, :], sps[:, off:off + OG])
                # dequant: w = (q - 7.5) * scale, scale broadcast 32-wide
                nc.vector.scalar_tensor_tensor(
                    w_tiles[t][:, :], qt[:, :], 7.5,
                    scomp[:, t, :, None].broadcast_to([128, OG, 32]),
                    op0=sub, op1=mult)

            def consume(t):
                for mt, ci in wgroups:
                    o0, on = chunks[ci]
                    nc.tensor.matmul(
                        wps[mt, ci][:, :on],
                        xtiles[mt][:, t, :],
                        w_tiles[t][:, o0:o0 + on],
                        start=(t == 0), stop=(t == kt - 1),
                    )

            bias_sb = consts.tile([128, O], f32)
            LA = 3  # expand() runs LA k-tiles ahead of consume()
            expand(0)
            warm(4)
            expand(1)
            # bias is only needed at the window close; issue its DMA now so
            # it is well clear of the critical startup transfers
            nc.sync.dma_start(out=bias_sb, in_=bb[:, :])
            expand(2)
            warm(4)
            for t in range(LA, kt):
                expand(t)
                consume(t - LA)
                # prefetch the first steady-state x m-tiles mid-window,
                # clear of both the startup transfers and the window close
                if t == 16:
                    xfetch(WMT)
                elif t == 24:
                    xfetch(WMT + 1)
            for t in range(kt - LA, kt):
                consume(t)

            # the left-out group runs on the spare bank right at the window
            # close, keeping the PE busy while the DVE drains the window banks
            lm, lc = WMT - 1, len(chunks) - 1
            lo0, lon = chunks[lc]
            lps = pp.tile([128, 512], f32, name="lps", tag="ps")
            for t in range(kt):
                nc.tensor.matmul(
                    lps[:, :lon],
                    xtiles[lm][:, t, :],
                    w_tiles[t][:, lo0:lo0 + lon],
                    start=(t == 0), stop=(t == kt - 1),
                )

            # close the window groups
            obs = {}
            for mt in range(WMT):
                obs[mt] = op.tile([128, O], f32, name=f"wob{mt}", tag="ob")
            for mt, ci in wgroups:
                o0, on = chunks[ci]
                nc.vector.tensor_add(
                    obs[mt][:, o0:o0 + on], wps[mt, ci][:, :on],
                    bias_sb[:, o0:o0 + on])
                nc.sync.dma_start(
                    out=y[mt * 128:(mt + 1) * 128, o0:o0 + on],
                    in_=obs[mt][:, o0:o0 + on])
            nc.vector.tensor_add(
                obs[lm][:, lo0:lo0 + lon], lps[:, :lon],
                bias_sb[:, lo0:lo0 + lon])
            nc.sync.dma_start(
                out=y[lm * 128:(lm + 1) * 128, lo0:lo0 + lon],
                in_=obs[lm][:, lo0:lo0 + lon])

            # ---- steady state ----
            n_mt = M // 128
            for j in range(WMT, n_mt):
                if j + 2 < n_mt:
                    xfetch(j + 2)
                xs = xtiles.pop(j)
                m0 = j * 128
                ob = op.tile([128, O], f32, name="ob", tag="ob")
                last_mt = j == n_mt - 1
                for ci, (o0, on) in enumerate(chunks):
                    ps = pp.tile([128, 512], f32, tag="ps")
                    for t in range(kt):
                        nc.tensor.matmul(
                            ps[:, :on],
                            xs[:, t, :],
                            w_tiles[t][:, o0:o0 + on],
                            start=(t == 0), stop=(t == kt - 1),
                        )
                    if last_mt and ci == len(chunks) - 1:
                        # split the very last close so the final DMA chain is
                        # short and overlaps the preceding adds
                        step = on // 4
                        for s0 in range(o0, o0 + on, step):
                            nc.vector.tensor_add(
                                ob[:, s0:s0 + step], ps[:, s0 - o0:s0 - o0 + step],
                                bias_sb[:, s0:s0 + step])
                            nc.sync.dma_start(
                                out=y[m0:m0 + 128, s0:s0 + step],
                                in_=ob[:, s0:s0 + step])
                    else:
                        nc.vector.tensor_add(
                            ob[:, o0:o0 + on], ps[:, :on], bias_sb[:, o0:o0 + on])
                        nc.sync.dma_start(
                            out=y[m0:m0 + 128, o0:o0 + on],
                            in_=ob[:, o0:o0 + on])
    nc.compile()
    return nc


def _get_prog(M=None, O=None, kt=None):
    key = (M or 8192, O or O_SH, kt or KT)
    if key not in _PROG:
        _PROG[key] = _build(*key)
    return _PROG[key]


def _in_maps(x, weight_q4, weight_norm, bias, n_cores=N_CORES):
    x = np.asarray(x)
    M = x.size // IN_F
    kt = IN_F // 128
    X = np.asarray(x, np.float32).reshape(M, IN_F).astype(np.float16)
    # (m-tile j, partition p, k-tile t, token q): per-partition-contiguous
    # slabs so each x DMA is 128 x 8KB descriptors
    xT = np.ascontiguousarray(
        X.reshape(M // 128, 128, kt, 128).transpose(0, 3, 2, 1))

    q = np.asarray(weight_q4).astype(np.uint8)          # (O, 128, 16)
    low = q & 15
    high = q >> 4
    w8 = np.stack((low, high), axis=-1).reshape(OUT_F, IN_F)
    wqT = np.ascontiguousarray(w8.T)                    # (4096, 11008) u8

    # compact scales: one column per 32-wide out-feature group
    nf = np.asarray(weight_norm, np.float32)[:, :, 0]   # (344, 128)
    s2cT = np.ascontiguousarray(
        (nf * (2.0 / 15.0)).T).astype(np.float16)       # (128, 344)

    bias = np.asarray(bias, np.float32)

    # one-hot matrices for on-device scale-row routing:
    # E_t[r, p] = 1 iff r == 4t + p//32  ->  (E_t.T @ s)[p, j] = s[4t+p//32, j]
    e_host = np.zeros((128, kt, 128), np.float16)
    p_idx = np.arange(128)
    for t in range(kt):
        e_host[4 * t + p_idx // 32, t, p_idx] = 1.0
    e_host = e_host.reshape(128, kt * 128)

    o_sh = OUT_F // n_cores
    og = o_sh // 32
    maps = []
    for c in range(n_cores):
        sl = slice(c * o_sh, (c + 1) * o_sh)
        maps.append({
            "xT": xT,
            "wq": np.ascontiguousarray(wqT[:, sl]),
            "s2c": np.ascontiguousarray(s2cT[:, c * og:(c + 1) * og]),
            "ee": e_host,
            "bb": np.ascontiguousarray(
                np.broadcast_to(bias[sl], (128, o_sh))),
        })
    return maps


def kernel(x, weight_q4, weight_norm, bias):
    from concourse.bass_utils import run_bass_kernel_spmd
    x = np.asarray(x)
    maps = _in_maps(x, weight_q4, weight_norm, bias)
    nc = _get_prog(M=x.size // IN_F)
    res = run_bass_kernel_spmd(nc, maps, core_ids=list(range(N_CORES)))
    og = O_SH // 32
    j = np.arange(O_SH)
    perm = (j % og) * 32 + (j // og)
    out = np.empty((x.size // IN_F, OUT_F), np.float32)
    for c, r in enumerate(res.results):
        out[:, c * O_SH + perm] = r["y"]
    return out.reshape(x.shape[0], x.shape[1], OUT_F)
